# revision 5
# baseline (speedup 1.0000x reference)
"""Trainium2 Bass kernel for nn_BERT_pool_mutil_avr (cosine-attention + ROI pool + conv).

Sharding: kernel 1 = (batch, T-half) per core; kernel 2 = 16 ROIs per core.
"""
import os
import numpy as np
import ml_dtypes

import concourse.bass as bass
import concourse.mybir as mybir
import concourse.tile as tile
from concourse import bacc, bass_utils
from concourse.masks import make_identity

TRACE = bool(int(os.environ.get("KTRACE", "0")))
LAST_EXEC_NS = 0
LAST_RES = []

F32 = mybir.dt.float32
F32R = mybir.dt.float32r
BF16 = mybir.dt.bfloat16
I32 = mybir.dt.int32
AF = mybir.ActivationFunctionType
OP = mybir.AluOpType

B, D, T, NROI, H, DK = 4, 1024, 2048, 128, 8, 128
SCALES = [1, 3, 7, 9]
NBT = 20                      # total bins per roi
OFF = [0, 1, 4, 11]           # bin offset of each scale
TH = T // 2                   # tokens per core in kernel 1
KT = D // 128                 # 8 contraction tiles
NPC = NROI // 8               # rois per core in kernel 2


def _chunks(total, maxc=512):
    nch = -(-total // maxc)
    base = -(-total // nch)
    out, s = [], 0
    while s < total:
        e = min(s + base, total)
        out.append((s, e - s))
        s = e
    return out


def build_k1(npad, has_bv):
    cols = npad * NBT
    cch = _chunks(cols)
    BF = mybir.dt.bfloat16
    nc = bacc.Bacc("TRN2", target_bir_lowering=False, debug=False, num_devices=8)
    xb = nc.dram_tensor("xb", [D, TH], BF16, kind="ExternalInput").ap()
    wq = nc.dram_tensor("wqT", [D, D], BF16, kind="ExternalInput").ap()
    wk = nc.dram_tensor("wkT", [D, D], BF16, kind="ExternalInput").ap()
    wv = nc.dram_tensor("wvT", [D, D], BF16, kind="ExternalInput").ap()
    clsb = nc.dram_tensor("clsb", [D, 1], BF16, kind="ExternalInput").ap()
    bqr = nc.dram_tensor("bqr", [1, D], F32, kind="ExternalInput").ap()
    bkc = nc.dram_tensor("bkc", [D, 1], F32, kind="ExternalInput").ap()
    bvr = nc.dram_tensor("bvr", [128, D], F32, kind="ExternalInput").ap() if has_bv else None
    roisp = nc.dram_tensor("roisp", [npad, 3], I32, kind="ExternalInput").ap()
    f1d = nc.dram_tensor("f1", [128, NBT], F32, kind="ExternalInput").ap()
    f2d = nc.dram_tensor("f2", [128, NBT], F32, kind="ExternalInput").ap()
    t0d = nc.dram_tensor("t0", [128, 1], F32, kind="ExternalInput").ap()
    pout = nc.dram_tensor("Pout", [D, cols], F32, kind="ExternalOutput").ap()
    cnto = nc.dram_tensor("cnt", [1, cols], F32, kind="ExternalOutput").ap()
    pout_r = pout.rearrange("(c p) l -> c p l", p=128)

    with tile.TileContext(nc) as tc:
        with (
            tc.tile_pool(name="const", bufs=1) as cp,
            tc.tile_pool(name="w", bufs=2) as wp,
            tc.tile_pool(name="big", bufs=1) as bigp,
            tc.tile_pool(name="k", bufs=3) as kp,
            tc.tile_pool(name="rows", bufs=1) as rp,
            tc.tile_pool(name="msk", bufs=1) as mp,
            tc.tile_pool(name="pev", bufs=2) as pp,
            tc.tile_pool(name="dram", bufs=1, space="DRAM") as dp,
            tc.tile_pool(name="psb", bufs=2, space="PSUM") as psb,
            tc.tile_pool(name="pss", bufs=4, space="PSUM") as pss,
            tc.tile_pool(name="pst", bufs=2, space="PSUM") as pst,
        ):
            ident = cp.tile([128, 128], F32)
            make_identity(nc, ident[:])
            ones_r = cp.tile([1, 128], F32)
            nc.gpsimd.memset(ones_r[:], 1.0)
            # consolidated small tiles
            cin = cp.tile([128, 64], F32)      # f1 0:20, f2 20:40, t0 40, cls 41:49, bk 49:57
            cw = cp.tile([128, 96], F32)       # q_ct 0:8, q2_ct 8:16, nq2_col 16, ones_cf 17, nq2_row r0 24:32, pT 32:96
            cm = cp.tile([128, 64], F32)       # tvf 0, tvg 1, tplus 2, tminus 3, lcol 4, bs_nf 8:28, be_nf 28:48
            ci = cp.tile([128, 4], I32)        # roi 0:3, tvi 3
            cb16 = cp.tile([128, 8], BF)       # ones_cb 0
            nc.sync.dma_start(cin[:, 0:20], f1d[:])
            nc.sync.dma_start(cin[:, 20:40], f2d[:])
            nc.sync.dma_start(cin[:, 40:41], t0d[:])
            cls_r = cp.tile([128, KT], BF16)
            nc.sync.dma_start(cls_r[:], clsb.rearrange("(k p) o -> p (k o)", p=128))
            nc.sync.dma_start(cin[:, 49:57], bkc.rearrange("(k p) o -> p (k o)", p=128))
            nc.gpsimd.memset(cw[:, 17:18], 1.0)
            nc.gpsimd.memset(cb16[:, 0:1], 1.0)

            x_sb = bigp.tile([128, KT, TH], BF16, tag="x")
            nc.sync.dma_start(x_sb[:], xb.rearrange("(k p) t -> p k t", p=128))
            w_q = wp.tile([128, KT, D], BF16, tag="w")
            nc.sync.dma_start(w_q[:], wq.rearrange("(k p) c -> p k c", p=128))
            bq_sb = rp.tile([1, D], F32)
            nc.sync.dma_start(bq_sb[:], bqr[:])
            bv_sb = None
            if has_bv:
                bv_sb = bigp.tile([128, D], F32, tag="bv")
                nc.sync.dma_start(bv_sb[:], bvr[:])

            # ---- roi masks  mask_sb [128, mt, cols] bf16
            nc.sync.dma_start(ci[:npad, 0:3], roisp[:])
            roif = cm[:npad, 5:8]
            nc.vector.tensor_copy(roif, ci[:npad, 0:3])
            nc.vector.tensor_sub(cm[:npad, 4:5], cm[:npad, 7:8], cm[:npad, 6:7])
            nc.vector.tensor_scalar(cm[:npad, 8:28], cin[0:npad, 0:20], cm[:npad, 4:5], None, op0=OP.mult)
            nc.vector.tensor_scalar_add(cm[:npad, 8:28], cm[:npad, 8:28], cm[:npad, 6:7])
            nc.vector.tensor_scalar(cm[:npad, 28:48], cin[0:npad, 20:40], cm[:npad, 4:5], None, op0=OP.mult)
            nc.vector.tensor_scalar_add(cm[:npad, 28:48], cm[:npad, 28:48], cm[:npad, 6:7])
            dbs = dp.tile([npad, NBT], F32)
            dbe = dp.tile([npad, NBT], F32)
            nc.sync.dma_start(dbs[:], cm[:npad, 8:28])
            nc.sync.dma_start(dbe[:], cm[:npad, 28:48])
            bs_row = rp.tile([1, cols], F32)
            be_row = rp.tile([1, cols], F32)
            nc.sync.dma_start(bs_row[:], dbs.rearrange("n i -> (n i)")[None, :])
            nc.sync.dma_start(be_row[:], dbe.rearrange("n i -> (n i)")[None, :])
            bs_bc = bigp.tile([128, cols], F32, tag="bsbc")
            be_bc = bigp.tile([128, cols], F32, tag="bebc")
            for s, w in cch:
                pb = psb.tile([128, 512], F32, tag="b")
                nc.tensor.matmul(pb[:, :w], ones_r[0:1, :], bs_row[0:1, s : s + w], start=True, stop=True)
                nc.scalar.activation(bs_bc[:, s : s + w], pb[:, :w], AF.Copy)
                pb2 = psb.tile([128, 512], F32, tag="b")
                nc.tensor.matmul(pb2[:, :w], ones_r[0:1, :], be_row[0:1, s : s + w], start=True, stop=True)
                nc.scalar.activation(be_bc[:, s : s + w], pb2[:, :w], AF.Copy)
            nc.gpsimd.iota(ci[:, 3:4], [[0, 1]], base=0, channel_multiplier=1)
            nc.vector.tensor_copy(cm[:, 0:1], ci[:, 3:4])
            nc.vector.tensor_add(cm[:, 1:2], cm[:, 0:1], cin[:, 40:41])
            mask_sb = bigp.tile([128, KT, cols], BF, tag="mask")
            for mt in range(KT):
                nc.vector.tensor_scalar_add(cm[:, 2:3], cm[:, 1:2], float(mt * 128) + 0.95)
                nc.vector.tensor_scalar_add(cm[:, 3:4], cm[:, 1:2], float(mt * 128) + 0.05)
                mtmp = mp.tile([128, cols], BF, tag="mtmp")
                nc.vector.tensor_scalar(mtmp[:], bs_bc[:], cm[:, 2:3], None, op0=OP.is_lt)
                nc.vector.tensor_scalar(mask_sb[:, mt, :], be_bc[:], cm[:, 3:4], None, op0=OP.is_gt)
                nc.vector.tensor_mul(mask_sb[:, mt, :], mask_sb[:, mt, :], mtmp[:])

            # ---- q projection: q_row [1, D]
            q_row = rp.tile([1, D], F32)
            for c2 in range(2):
                ps = pss.tile([1, 512], F32, tag="s")
                for k in range(KT):
                    nc.tensor.matmul(ps[:], cls_r[:, k : k + 1], w_q[:, k, c2 * 512 : (c2 + 1) * 512],
                                     start=(k == 0), stop=(k == KT - 1))
                nc.vector.tensor_add(q_row[0:1, c2 * 512 : (c2 + 1) * 512], ps[:], bq_sb[0:1, c2 * 512 : (c2 + 1) * 512])
            for ct in range(H):
                tp = pst.tile([128, 8], F32, tag="tr")
                nc.tensor.transpose(tp[:, 0:1], q_row[0:1, ct * 128 : (ct + 1) * 128], ident[0:1, 0:1])
                nc.scalar.activation(cw[:, ct : ct + 1], tp[:, 0:1], AF.Copy)
            nc.scalar.activation(cw[:, 8:16], cw[:, 0:8], AF.Square)
            nqp = pst.tile([128, 8], F32, tag="tr")
            nc.tensor.matmul(nqp[0:1, :], cw[:, 17:18], cw[:, 8:16], start=True, stop=True)
            nc.scalar.activation(cw[0:1, 24:32], nqp[0:1, :], AF.Copy)
            nqc = pst.tile([128, 8], F32, tag="tr")
            nc.tensor.transpose(nqc[0:8, 0:1], cw[0:1, 24:32], ident[0:1, 0:1])
            nc.scalar.activation(cw[0:8, 16:17], nqc[0:8, 0:1], AF.Copy)
            # masked lhsT blocks: qz col ct*8+h = q (h==ct) else 0; onesz likewise
            cw2 = cp.tile([128, 128], F32R)
            nc.scalar.activation(cw2[:], ident[:], AF.Copy, scale=0.0)
            for ct in range(H):
                nc.scalar.activation(cw2[:, ct * 8 + ct : ct * 8 + ct + 1], cw[:, ct : ct + 1], AF.Copy)
                nc.scalar.activation(cw2[:, 64 + ct * 8 + ct : 64 + ct * 8 + ct + 1], cw[:, 17:18], AF.Copy)

            # ---- K projection + dot + nk2 per head
            w_k = wp.tile([128, KT, D], BF16, tag="w")
            nc.sync.dma_start(w_k[:], wk.rearrange("(k p) c -> p k c", p=128))
            dot_sb = rp.tile([H, TH], F32)
            nk2_sb = rp.tile([H, TH], F32)
            psd_all = [pss.tile([H, 512], F32, tag="s", name=f"psd{i}") for i in range(2)]
            psn_all = [pss.tile([H, 512], F32, tag="s", name=f"psn{i}") for i in range(2)]
            for ct in range(H):
                ktile = kp.tile([128, TH], F32R, tag="k")
                for c2 in range(2):
                    ps = psb.tile([128, 512], F32, tag="b")
                    for k in range(KT):
                        nc.tensor.matmul(ps[:], w_k[:, k, ct * 128 : (ct + 1) * 128], x_sb[:, k, c2 * 512 : (c2 + 1) * 512],
                                         start=(k == 0), stop=(k == KT - 1))
                    nc.scalar.activation(ktile[:, c2 * 512 : (c2 + 1) * 512], ps[:], AF.Identity, bias=cin[:, 49 + ct : 50 + ct])
                k2t = kp.tile([128, TH], F32R, tag="k")
                nc.scalar.activation(k2t[:], ktile[:], AF.Square)
                for c2 in range(2):
                    sl = slice(c2 * 512, (c2 + 1) * 512)
                    nc.tensor.matmul(psd_all[c2][:], cw2[:, ct * 8 : ct * 8 + 8], ktile[:, sl],
                                     start=(ct == 0), stop=(ct == H - 1))
                    nc.tensor.matmul(psn_all[c2][:], cw2[:, 64 + ct * 8 : 64 + ct * 8 + 8], k2t[:, sl],
                                     start=(ct == 0), stop=(ct == H - 1))
            for c2 in range(2):
                sl = slice(c2 * 512, (c2 + 1) * 512)
                nc.vector.tensor_copy(dot_sb[:, sl], psd_all[c2][:])
                nc.vector.tensor_copy(nk2_sb[:, sl], psn_all[c2][:])

            # ---- attention probs p (in place over dot_sb)
            nc.vector.tensor_scalar(nk2_sb[:], nk2_sb[:], cw[0:8, 16:17], None, op0=OP.mult)
            nc.vector.tensor_scalar_max(nk2_sb[:], nk2_sb[:], 1e-16)
            nc.scalar.activation(nk2_sb[:], nk2_sb[:], AF.Sqrt)
            nc.vector.reciprocal(nk2_sb[:], nk2_sb[:])
            nc.vector.tensor_mul(dot_sb[:], dot_sb[:], nk2_sb[:])
            nc.vector.tensor_scalar_mul(nk2_sb[:], dot_sb[:], -1.0)
            nc.vector.tensor_max(dot_sb[:], dot_sb[:], nk2_sb[:])
            nc.scalar.activation(dot_sb[:], dot_sb[:], AF.Exp)
            for mt in range(H):
                tp = pst.tile([128, 8], F32, tag="tr")
                nc.tensor.transpose(tp[:], dot_sb[0:H, mt * 128 : (mt + 1) * 128], ident[0:H, 0:H])
                nc.scalar.activation(cw[:, 32 + mt * H : 32 + (mt + 1) * H], tp[:], AF.Copy)

            # ---- V projection + val = p * v   (val_sb [128, mt, c] bf16)
            w_v = wp.tile([128, KT, D], BF16, tag="w")
            nc.sync.dma_start(w_v[:], wv.rearrange("(k p) c -> p k c", p=128))
            val_sb = bigp.tile([128, KT, D], BF, tag="val")
            for mt in range(KT):
                for c2 in range(2):
                    ps = psb.tile([128, 512], F32, tag="b")
                    for k in range(KT):
                        nc.tensor.matmul(ps[:], x_sb[:, k, mt * 128 : (mt + 1) * 128], w_v[:, k, c2 * 512 : (c2 + 1) * 512],
                                         start=(k == 0), stop=(k == KT - 1))
                    for hl in range(4):
                        h = c2 * 4 + hl
                        src = ps[:, hl * 128 : (hl + 1) * 128]
                        dst = val_sb[:, mt, c2 * 512 + hl * 128 : c2 * 512 + (hl + 1) * 128]
                        pcol = cw[:, 32 + mt * H + h : 32 + mt * H + h + 1]
                        if has_bv:
                            tmp = kp.tile([128, 128], F32, tag="bvtmp")
                            nc.vector.tensor_add(tmp[:], src, bv_sb[:, c2 * 512 + hl * 128 : c2 * 512 + (hl + 1) * 128])
                            nc.vector.tensor_scalar(dst, tmp[:], pcol, None, op0=OP.mult)
                        else:
                            nc.vector.tensor_scalar(dst, src, pcol, None, op0=OP.mult)

            # ---- pooling partials P[c, col] and counts
            for ct in range(H):
                ptile = pp.tile([128, cols], F32, tag="pout")
                for s, w in cch:
                    ps = psb.tile([128, 512], F32, tag="b")
                    for mt in range(KT):
                        nc.tensor.matmul(ps[:, :w], val_sb[:, mt, ct * 128 : (ct + 1) * 128], mask_sb[:, mt, s : s + w],
                                         start=(mt == 0), stop=(mt == KT - 1))
                    nc.scalar.activation(ptile[:, s : s + w], ps[:, :w], AF.Copy)
                nc.sync.dma_start(pout_r[ct], ptile[:])
            cnt_row = rp.tile([1, cols], F32)
            for s, w in cch:
                ps = pss.tile([1, 512], F32, tag="s")
                for mt in range(KT):
                    nc.tensor.matmul(ps[:, :w], cb16[:, 0:1], mask_sb[:, mt, s : s + w],
                                     start=(mt == 0), stop=(mt == KT - 1))
                nc.vector.tensor_copy(cnt_row[0:1, s : s + w], ps[:, :w])
            nc.sync.dma_start(cnto[:], cnt_row[:])

    nc.compile()
    return nc


def build_k2():
    cols = NPC * NBT  # 320
    nc = bacc.Bacc("TRN2", target_bir_lowering=False, debug=False, num_devices=8)
    pa = nc.dram_tensor("Pa", [D, cols], F32, kind="ExternalInput").ap()
    pb = nc.dram_tensor("Pb", [D, cols], F32, kind="ExternalInput").ap()
    ca = nc.dram_tensor("ca", [1, cols], F32, kind="ExternalInput").ap()
    cb = nc.dram_tensor("cb", [1, cols], F32, kind="ExternalInput").ap()
    wts = {nb: nc.dram_tensor(f"wt{nb}", [2 * nb * 128, 256], F32R, kind="ExternalInput").ap() for nb in SCALES}
    cbias = nc.dram_tensor("cbias", [NPC, D], F32, kind="ExternalInput").ap()
    out = nc.dram_tensor("out", [NPC, D], F32, kind="ExternalOutput").ap()

    with tile.TileContext(nc) as tc:
        with (
            tc.tile_pool(name="io", bufs=1) as iop,
            tc.tile_pool(name="wt", bufs=2) as wtp,
            tc.tile_pool(name="sm", bufs=1) as smp,
            tc.tile_pool(name="psc", bufs=2, space="PSUM") as psc,
            tc.tile_pool(name="psb", bufs=2, space="PSUM") as psb,
        ):
            ones_r = smp.tile([1, 128], F32)
            nc.gpsimd.memset(ones_r[:], 1.0)
            pa_sb = iop.tile([128, H, cols], F32, tag="pa")
            pb_sb = iop.tile([128, H, cols], F32, tag="pb")
            nc.sync.dma_start(pa_sb[:], pa.rearrange("(c p) l -> p c l", p=128))
            nc.sync.dma_start(pb_sb[:], pb.rearrange("(c p) l -> p c l", p=128))
            ca_sb = smp.tile([1, cols], F32)
            cb_sb = smp.tile([1, cols], F32)
            nc.sync.dma_start(ca_sb[:], ca[:])
            nc.sync.dma_start(cb_sb[:], cb[:])
            cbias_sb = smp.tile([NPC, D], F32)
            nc.sync.dma_start(cbias_sb[:], cbias[:])
            wt_sb = {}
            for nb in SCALES:
                wt_sb[nb] = wtp.tile([128, 2 * nb, 256], F32R, tag=f"wt{nb}", name=f"wt{nb}_sb")
                nc.sync.dma_start(wt_sb[nb][:], wts[nb].rearrange("(c i p) o -> p (c i) o", p=128, i=nb))

            nc.vector.tensor_add(pa_sb[:], pa_sb[:], pb_sb[:])
            csum = smp.tile([1, cols], F32)
            nc.vector.tensor_add(csum[:], ca_sb[:], cb_sb[:])
            nc.vector.tensor_scalar_max(csum[:], csum[:], 1.0)
            inv = smp.tile([1, cols], F32)
            nc.vector.reciprocal(inv[:], csum[:])
            pbc = psb.tile([128, cols], F32, tag="bc")
            nc.tensor.matmul(pbc[:], ones_r[0:1, :], inv[0:1, :], start=True, stop=True)
            inv_bc = smp.tile([128, cols], F32)
            nc.scalar.activation(inv_bc[:], pbc[:], AF.Copy)
            pn_sb = iop.tile([128, H, cols], F32R, tag="pn")
            for ct in range(H):
                nc.vector.tensor_mul(pn_sb[:, ct, :], pa_sb[:, ct, :], inv_bc[:])

            pa_r = pn_sb.rearrange("p c (n i) -> p c n i", i=NBT)
            out_sb = smp.tile([NPC, D], F32)
            for j, nb in enumerate(SCALES):
                po = psc.tile([NPC, 256], F32, tag="o")
                mms = [(ctl, i) for ctl in range(2) for i in range(nb)]
                for idx, (ctl, i) in enumerate(mms):
                    ct = 2 * j + ctl
                    lhsT = pa_r[:, ct, :, OFF[j] + i]
                    nc.tensor.matmul(po[:], lhsT, wt_sb[nb][:, ctl * nb + i, :],
                                     start=(idx == 0), stop=(idx == len(mms) - 1))
                nc.vector.tensor_add(out_sb[:, j * 256 : (j + 1) * 256], po[:], cbias_sb[:, j * 256 : (j + 1) * 256])
            nc.sync.dma_start(out[:], out_sb[:])

    nc.compile()
    return nc


def kernel(**inputs):
    global LAST_EXEC_NS, LAST_RES
    LAST_EXEC_NS = 0
    LAST_RES = []
    iv = np.asarray(inputs["input_vectors"], np.float32)
    cls = np.asarray(inputs["clstoken_scales"], np.float32)
    rois = np.asarray(inputs["rois"], np.int32)
    wqT = np.ascontiguousarray(np.asarray(inputs["Wq"], np.float32).T)
    wkT = np.ascontiguousarray(np.asarray(inputs["Wk"], np.float32).T)
    wvT = np.ascontiguousarray(np.asarray(inputs["Wv"], np.float32).T)
    bq = np.asarray(inputs["bq"], np.float32)
    bk = np.asarray(inputs["bk"], np.float32)
    bv = np.asarray(inputs["bv"], np.float32)
    has_bv = bool(np.any(bv))

    wqT16 = wqT.astype(ml_dtypes.bfloat16)
    wkT16 = wkT.astype(ml_dtypes.bfloat16)
    wvT16 = wvT.astype(ml_dtypes.bfloat16)
    order = np.argsort(rois[:, 0], kind="stable")
    rs = rois[order]
    starts, counts = [], []
    for b in range(B):
        idx = np.nonzero(rs[:, 0] == b)[0]
        starts.append(int(idx[0]) if len(idx) else 0)
        counts.append(len(idx))
    npad = max(max(counts), 1)
    padded = []
    for b in range(B):
        arr = np.zeros((npad, 3), np.int32)
        arr[:, 2] = 16
        if counts[b]:
            arr[: counts[b]] = rs[starts[b] : starts[b] + counts[b]]
        padded.append(arr)

    f1 = np.zeros(NBT, np.float32)
    f2 = np.zeros(NBT, np.float32)
    for j, nb in enumerate(SCALES):
        for i in range(nb):
            f1[OFF[j] + i] = i / nb
            f2[OFF[j] + i] = (i + 1) / nb
    f1r = np.ascontiguousarray(np.broadcast_to(f1, (128, NBT)))
    f2r = np.ascontiguousarray(np.broadcast_to(f2, (128, NBT)))

    nc1 = build_k1(npad, has_bv)
    in1 = []
    for core in range(8):
        b, half = core // 2, core % 2
        m = {
            "xb": np.ascontiguousarray(iv[b, :, half * TH : (half + 1) * TH]).astype(ml_dtypes.bfloat16),
            "wqT": wqT16, "wkT": wkT16, "wvT": wvT16,
            "clsb": np.ascontiguousarray(cls[b][:, None]).astype(ml_dtypes.bfloat16),
            "bqr": np.ascontiguousarray(bq[None, :]),
            "bkc": np.ascontiguousarray(bk[:, None]),
            "roisp": padded[b],
            "f1": f1r, "f2": f2r,
            "t0": np.full((128, 1), half * TH, np.float32),
        }
        if has_bv:
            m["bvr"] = np.ascontiguousarray(np.broadcast_to(bv, (128, D)))
        in1.append(m)
    r1 = bass_utils.run_bass_kernel_spmd(nc1, in1, core_ids=list(range(8)), trace=TRACE)
    if r1.exec_time_ns:
        LAST_EXEC_NS += r1.exec_time_ns
    LAST_RES.append(r1)
    phalf = [r1.results[c]["Pout"] for c in range(8)]
    chalf = [r1.results[c]["cnt"] for c in range(8)]

    wt_in = {}
    for j, nb in enumerate(SCALES):
        cw = np.asarray(inputs[f"conv_w{nb}"], np.float32)          # [o, c, i]
        a = cw.transpose(1, 2, 0).reshape(2, 128, nb, 256)          # [ct, p, i, o]
        wt_in[nb] = np.ascontiguousarray(a.transpose(0, 2, 1, 3).reshape(2 * nb * 128, 256))
    cbias = np.concatenate([np.asarray(inputs[f"conv_b{nb}"], np.float32) for nb in SCALES])
    cbias_r = np.ascontiguousarray(np.broadcast_to(cbias, (NPC, D)))

    nc2 = build_k2()
    in2 = []
    for core in range(8):
        pa = np.empty((D, NPC * NBT), np.float32)
        pb = np.empty((D, NPC * NBT), np.float32)
        ca = np.empty((1, NPC * NBT), np.float32)
        cb = np.empty((1, NPC * NBT), np.float32)
        for r in range(NPC):
            g = core * NPC + r
            b = int(rs[g, 0])
            pos = g - starts[b]
            sl_src = slice(pos * NBT, (pos + 1) * NBT)
            sl_dst = slice(r * NBT, (r + 1) * NBT)
            pa[:, sl_dst] = phalf[2 * b][:, sl_src]
            pb[:, sl_dst] = phalf[2 * b + 1][:, sl_src]
            ca[:, sl_dst] = chalf[2 * b][:, sl_src]
            cb[:, sl_dst] = chalf[2 * b + 1][:, sl_src]
        m = {"Pa": pa, "Pb": pb, "ca": ca, "cb": cb, "cbias": cbias_r}
        for nb in SCALES:
            m[f"wt{nb}"] = wt_in[nb]
        in2.append(m)
    r2 = bass_utils.run_bass_kernel_spmd(nc2, in2, core_ids=list(range(8)), trace=TRACE)
    if r2.exec_time_ns:
        LAST_EXEC_NS += r2.exec_time_ns
    LAST_RES.append(r2)
    stacked = np.concatenate([r2.results[c]["out"] for c in range(8)], axis=0)
    final = np.empty((NROI, D), np.float32)
    final[order] = stacked
    return final



# revision 17
# speedup vs baseline: 1.3192x; 1.3192x over previous
"""Trainium2 Bass kernel for nn_BERT_pool_mutil_avr (cosine-attention + ROI pool + conv).

Single fused launch. Sharding: core pair (2b, 2b+1) both take batch b with the
full T=2048 tokens; core 2b owns heads {0,1,6,7} (conv scales nb=1,9), core
2b+1 owns heads {2,3,4,5} (scales nb=3,7). The conv is block-diagonal over
scale groups, so each core pools and convolves its ROIs completely locally —
no cross-core exchange. 10 pooling bins per ROI on every core (balanced).
The conv enumerates all (head, bin) pairs with zero-padded weights so the
instruction stream is identical on every core (SPMD); the per-core scale
structure lives purely in the data.
"""
import os
import numpy as np
import ml_dtypes

import concourse.bass as bass
import concourse.mybir as mybir
import concourse.tile as tile
from concourse import bacc, bass_utils
from concourse.masks import make_identity

F32 = mybir.dt.float32
F32R = mybir.dt.float32r
BF16 = mybir.dt.bfloat16
I32 = mybir.dt.int32
AF = mybir.ActivationFunctionType
OP = mybir.AluOpType

TRACE = bool(int(os.environ.get("KTRACE", "0")))
LAST_EXEC_NS = 0
LAST_RES = []

B, D, T, NROI, H, DK = 4, 1024, 2048, 128, 8, 128
KT = D // 128          # 8 contraction tiles
MT = T // 128          # 16 token tiles
C2 = T // 512          # 4 moving chunks
HC = 4                 # local heads per core
CH = HC * DK           # 512 local channels
NBK = 10               # pooling bins per roi per core
HSETS = [[0, 1, 6, 7], [2, 3, 4, 5]]
SCK = [[(1, 0), (9, 1)], [(3, 0), (7, 3)]]   # (nb, local bin offset) per half-set
OUTCOL = [[(0, 256), (768, 1024)], [(256, 512), (512, 768)]]


def _chunks(total, maxc=512):
    nch = -(-total // maxc)
    base = -(-total // nch)
    out, s = [], 0
    while s < total:
        e = min(s + base, total)
        out.append((s, e - s))
        s = e
    return out


def build_fused(npad, has_bq, has_bk, has_bv):
    cols = npad * NBK
    cch = _chunks(cols)
    nc = bacc.Bacc("TRN2", target_bir_lowering=False, debug=False, num_devices=8)
    xb = nc.dram_tensor("xb", [D, T], BF16, kind="ExternalInput").ap()
    wq = nc.dram_tensor("wqh", [D, CH], BF16, kind="ExternalInput").ap()
    wk = nc.dram_tensor("wkh", [D, CH], BF16, kind="ExternalInput").ap()
    wv = nc.dram_tensor("wvh", [D, CH], BF16, kind="ExternalInput").ap()
    clsb = nc.dram_tensor("clsb", [D, 1], BF16, kind="ExternalInput").ap()
    bsr = nc.dram_tensor("bsr", [1, cols], F32, kind="ExternalInput").ap()
    ber = nc.dram_tensor("ber", [1, cols], F32, kind="ExternalInput").ap()
    invr = nc.dram_tensor("invr", [1, cols], F32, kind="ExternalInput").ap()
    wtd = nc.dram_tensor("wt", [4 * NBK * 128, 256], BF16, kind="ExternalInput").ap()
    cbh = nc.dram_tensor("cbh", [npad, CH], F32, kind="ExternalInput").ap()
    bqr = nc.dram_tensor("bqr", [1, CH], F32, kind="ExternalInput").ap() if has_bq else None
    bkc = nc.dram_tensor("bkc", [128, HC], F32, kind="ExternalInput").ap() if has_bk else None
    bvr = nc.dram_tensor("bvr", [128, CH], F32, kind="ExternalInput").ap() if has_bv else None
    out = nc.dram_tensor("out", [npad, CH], F32, kind="ExternalOutput").ap()

    with tile.TileContext(nc) as tc:
        with (
            tc.tile_pool(name="const", bufs=1) as cp,
            tc.tile_pool(name="big", bufs=1) as bigp,
            tc.tile_pool(name="w", bufs=1) as wp,
            tc.tile_pool(name="kc", bufs=4) as kp,
            tc.tile_pool(name="mtp", bufs=2) as mp,
            tc.tile_pool(name="kv", bufs=3, space="PSUM") as kvp,
            tc.tile_pool(name="dn", bufs=2, space="PSUM") as dnp,
            tc.tile_pool(name="tr", bufs=1, space="PSUM") as trp,
            tc.tile_pool(name="po", bufs=2, space="PSUM") as pop,
        ):
            # ---- small DMAs on the scalar HWDGE ring
            bs_row = cp.tile([1, cols], F32)
            be_row = cp.tile([1, cols], F32)
            inv_row = cp.tile([1, cols], F32)
            nc.scalar.dma_start(bs_row[:], bsr[:])
            nc.scalar.dma_start(be_row[:], ber[:])
            nc.scalar.dma_start(inv_row[:], invr[:])
            cls_r = cp.tile([128, KT], BF16)
            nc.scalar.dma_start(cls_r[:], clsb.rearrange("(k p) o -> p (k o)", p=128))
            cb_sb = cp.tile([npad, CH], F32)
            nc.scalar.dma_start(cb_sb[:], cbh[:])
            bq_sb = bk_sb = bv_sb = None
            if has_bq:
                bq_sb = cp.tile([1, CH], F32)
                nc.scalar.dma_start(bq_sb[:], bqr[:])
            if has_bk:
                bk_sb = cp.tile([128, HC], F32)
                nc.scalar.dma_start(bk_sb[:], bkc[:])
            if has_bv:
                bv_sb = cp.tile([128, CH], F32)
                nc.scalar.dma_start(bv_sb[:], bvr[:])

            # ---- big DMAs on the sync HWDGE ring (issue order matters)
            w_q = wp.tile([128, KT, CH], BF16, tag="wq")
            nc.sync.dma_start(w_q[:], wq.rearrange("(k p) c -> p k c", p=128))
            x_sb = bigp.tile([128, KT, T], BF16, tag="x")
            x_r = xb.rearrange("(k p) t -> p k t", p=128)
            nc.sync.dma_start(x_sb[:, :, 0:512], x_r[:, :, 0:512])
            w_k = wp.tile([128, KT, CH], BF16, tag="wk")
            nc.sync.dma_start(w_k[:], wk.rearrange("(k p) c -> p k c", p=128))
            for c2 in range(1, C2):
                nc.sync.dma_start(
                    x_sb[:, :, c2 * 512 : (c2 + 1) * 512], x_r[:, :, c2 * 512 : (c2 + 1) * 512]
                )
            w_v = wp.tile([128, KT, CH], BF16, tag="wv")
            nc.sync.dma_start(w_v[:], wv.rearrange("(k p) c -> p k c", p=128))
            wt_sb = wp.tile([128, 4 * NBK, 256], BF16, tag="wt")
            nc.sync.dma_start(wt_sb[:], wtd.rearrange("(w p) o -> p w o", p=128))

            ident = cp.tile([128, 128], F32)
            make_identity(nc, ident[:])
            ones_r = cp.tile([1, 128], F32)
            nc.gpsimd.memset(ones_r[:], 1.0)
            onesf = cp.tile([128, 1], F32)
            nc.gpsimd.memset(onesf[:], 1.0)
            ones_c = cp.tile([128, 1], F32R)
            nc.scalar.activation(ones_c[:], onesf[:], AF.Copy)

            # ---- broadcast bs/be/inv to 128 partitions (PE, early + tiny)
            bs_bc = bigp.tile([128, cols], F32, tag="bsbc")
            be_bc = bigp.tile([128, cols], F32, tag="bebc")
            inv_bc = bigp.tile([128, cols], F32, tag="invbc")
            for row, dst in ((bs_row, bs_bc), (be_row, be_bc), (inv_row, inv_bc)):
                for s, w in cch:
                    pb = kvp.tile([128, 512], F32, tag="kv")
                    nc.tensor.matmul(pb[:, :w], ones_r[0:1, :], row[0:1, s : s + w], start=True, stop=True)
                    nc.scalar.activation(dst[:, s : s + w], pb[:, :w], AF.Copy)

            # ---- q projection -> masked lhsT blocks (col 2ct = qcol / 2ct+1 = 1)
            #      cw[:, ct*16 : ct*16+8] = q-mask, [+8:+16] = ones-mask,
            #      cw[:, 64:72] = q2col8; scl8 [8,1] = (1, nq2) x 4
            cw = cp.tile([128, 72], F32R)
            nc.scalar.activation(cw[:, 0:64], ident[:, 0:64], AF.Copy, scale=0.0)
            psq = kvp.tile([128, 512], F32, tag="kv")
            for k in range(KT):
                nc.tensor.matmul(psq[0:1, :], cls_r[:, k : k + 1], w_q[:, k, :], start=(k == 0), stop=(k == KT - 1))
            q_row = cp.tile([1, CH], F32)
            if has_bq:
                nc.vector.tensor_add(q_row[:], psq[0:1, :], bq_sb[:])
            else:
                nc.vector.tensor_copy(q_row[:], psq[0:1, :])
            for ct in range(HC):
                tp = trp.tile([128, 8], F32, tag="tr")
                nc.tensor.transpose(tp[:, 0:1], q_row[0:1, ct * 128 : (ct + 1) * 128], ident[0:1, 0:1])
                nc.scalar.activation(cw[:, ct * 16 + 2 * ct : ct * 16 + 2 * ct + 1], tp[:, 0:1], AF.Copy)
                nc.scalar.activation(cw[:, 64 + 2 * ct + 1 : 64 + 2 * ct + 2], tp[:, 0:1], AF.Square)
                nc.scalar.activation(cw[:, ct * 16 + 8 + 2 * ct + 1 : ct * 16 + 8 + 2 * ct + 2], onesf[:], AF.Copy)
                nc.scalar.activation(cw[:, 64 + 2 * ct : 64 + 2 * ct + 1], onesf[:], AF.Copy, scale=1.0 / 128.0)
            nqp = trp.tile([128, 8], F32, tag="tr")
            nc.tensor.matmul(nqp[0:1, :], ones_c[:, 0:1], cw[:, 64:72], start=True, stop=True)
            sc_row = cp.tile([1, 8], F32)
            nc.scalar.activation(sc_row[:], nqp[0:1, :], AF.Copy)
            nqc = trp.tile([128, 8], F32, tag="tr")
            nc.tensor.transpose(nqc[0:8, 0:1], sc_row[0:1, :], ident[0:1, 0:1])
            scl8 = cp.tile([8, 1], F32)
            nc.scalar.activation(scl8[:], nqc[0:8, 0:1], AF.Copy)

            # ---- masks: mask[t, col] = (t>=bs)&(t<be) * (1/cnt)   bf16
            tvi = cp.tile([128, 1], I32)
            nc.gpsimd.iota(tvi[:], [[0, 1]], base=0, channel_multiplier=1)
            tv = cp.tile([128, 4], F32)  # 0: iota, 2: thr_hi, 3: thr_lo
            nc.vector.tensor_copy(tv[:, 0:1], tvi[:])
            mask_sb = bigp.tile([128, MT, cols], BF16, tag="mask")
            for mt in range(MT):
                nc.vector.tensor_scalar_add(tv[:, 2:3], tv[:, 0:1], float(mt * 128) + 0.95)
                nc.vector.tensor_scalar_add(tv[:, 3:4], tv[:, 0:1], float(mt * 128) + 0.05)
                mtmp = mp.tile([128, cols], F32, tag="mtmp")
                nc.vector.scalar_tensor_tensor(mtmp[:], be_bc[:], tv[:, 3:4], inv_bc[:], op0=OP.is_gt, op1=OP.mult)
                nc.vector.scalar_tensor_tensor(mask_sb[:, mt, :], bs_bc[:], tv[:, 2:3], mtmp[:], op0=OP.is_lt, op1=OP.mult)

            # ---- K projection + dot/nk2 rows (dnk [8, MT, 128] f32)
            dnk = bigp.tile([8, MT, 128], F32, tag="dnk")
            dnk_c = dnk.rearrange("p m t -> p (m t)")
            for c2 in range(C2):
                dn8 = dnp.tile([8, 512], F32, tag="dn")
                for ct in range(HC):
                    ps = kvp.tile([128, 512], F32, tag="kv")
                    for k in range(KT):
                        nc.tensor.matmul(
                            ps[:], w_k[:, k, ct * 128 : (ct + 1) * 128], x_sb[:, k, c2 * 512 : (c2 + 1) * 512],
                            start=(k == 0), stop=(k == KT - 1),
                        )
                    ktile = kp.tile([128, 512], F32R, tag="kc")
                    k2t = kp.tile([128, 512], F32R, tag="kc")
                    if has_bk:
                        nc.scalar.activation(ktile[:], ps[:], AF.Identity, bias=bk_sb[:, ct : ct + 1])
                        nc.scalar.activation(k2t[:], ktile[:], AF.Square)
                    else:
                        nc.scalar.activation(ktile[:], ps[:], AF.Copy)
                        nc.scalar.activation(k2t[:], ps[:], AF.Square)
                    nc.tensor.matmul(dn8[:], cw[:, ct * 16 : ct * 16 + 8], ktile[:], start=(ct == 0), stop=False)
                    nc.tensor.matmul(dn8[:], cw[:, ct * 16 + 8 : ct * 16 + 16], k2t[:], start=False, stop=(ct == HC - 1))
                # even rows: dot (x1), odd rows: nk2 (x nq2) via scl8
                nc.vector.tensor_scalar(
                    dnk_c[:, c2 * 512 : (c2 + 1) * 512], dn8[:], scl8[:, 0:1], None, op0=OP.mult
                )

            # ---- transpose to columns; probs math in column space
            dcol = bigp.tile([128, MT, 8], F32, tag="dcol")
            for mt in range(MT):
                tp = trp.tile([128, 8], F32, tag="tr")
                nc.tensor.transpose(tp[:], dnk[:, mt, :], ident[0:8, 0:8])
                nc.scalar.activation(dcol[:, mt, :], tp[:], AF.Copy)
            Dv = dcol[:, :, 0:8:2]
            Nv = dcol[:, :, 1:8:2]
            pcol = cp.tile([128, MT, HC], F32)
            nc.vector.tensor_scalar_max(Nv, Nv, 1e-16)
            nc.scalar.activation(Nv, Nv, AF.Sqrt)
            nc.vector.reciprocal(Nv, Nv)
            nc.vector.tensor_mul(Dv, Dv, Nv)                       # cos
            nc.vector.tensor_scalar_mul(pcol[:], Dv, -1.0)
            nc.vector.tensor_max(pcol[:], pcol[:], Dv)             # |cos|
            nc.scalar.activation(pcol[:], pcol[:], AF.Exp)

            # ---- V projection + val = p*v
            val_sb = bigp.tile([128, MT, CH], BF16, tag="val")
            for mt in range(MT):
                psv = kvp.tile([128, 512], F32, tag="kv")
                for k in range(KT):
                    nc.tensor.matmul(
                        psv[:], x_sb[:, k, mt * 128 : (mt + 1) * 128], w_v[:, k, :],
                        start=(k == 0), stop=(k == KT - 1),
                    )
                src = psv
                if has_bv:
                    tmpv = kp.tile([128, 512], F32, tag="bvtmp")
                    nc.vector.tensor_add(tmpv[:], psv[:], bv_sb[:])
                    src = tmpv
                for h in range(HC):
                    if h % 2 == 0:
                        nc.vector.tensor_scalar(
                            val_sb[:, mt, h * 128 : (h + 1) * 128], src[:, h * 128 : (h + 1) * 128],
                            pcol[:, mt, h : h + 1], None, op0=OP.mult,
                        )
                    else:
                        nc.scalar.activation(
                            val_sb[:, mt, h * 128 : (h + 1) * 128], src[:, h * 128 : (h + 1) * 128],
                            AF.Copy, scale=pcol[:, mt, h : h + 1],
                        )

            # ---- pooling P[ct] = sum_t val * mask   [128, cols] bf16
            p_sb = bigp.tile([128, HC, cols], BF16, tag="psb")
            for ct in range(HC):
                for s, w in cch:
                    pk = kvp.tile([128, 512], F32, tag="kv")
                    for mt in range(MT):
                        nc.tensor.matmul(
                            pk[:, :w], val_sb[:, mt, ct * 128 : (ct + 1) * 128], mask_sb[:, mt, s : s + w],
                            start=(mt == 0), stop=(mt == MT - 1),
                        )
                    nc.scalar.activation(p_sb[:, ct, s : s + w], pk[:, :w], AF.Copy)

            # ---- conv: uniform over all (jl, ctl, bin) with zero-padded wt
            p_r = p_sb.rearrange("p c (n i) -> p c n i", i=NBK)
            out_sb = cp.tile([npad, CH], F32)
            for jl in range(2):
                po = pop.tile([npad, 256], F32, tag="po")
                mms = [(ctl, i) for ctl in range(2) for i in range(NBK)]
                for idx, (ctl, i) in enumerate(mms):
                    ct = 2 * jl + ctl
                    nc.tensor.matmul(
                        po[:], p_r[:, ct, :, i], wt_sb[:, jl * 2 * NBK + ctl * NBK + i, :],
                        start=(idx == 0), stop=(idx == len(mms) - 1),
                    )
                nc.vector.tensor_add(out_sb[:, jl * 256 : (jl + 1) * 256], po[:], cb_sb[:, jl * 256 : (jl + 1) * 256])
            nc.sync.dma_start(out[:], out_sb[:])

    nc.compile()
    return nc


def kernel(**inputs):
    global LAST_EXEC_NS, LAST_RES
    LAST_EXEC_NS = 0
    LAST_RES = []
    iv = np.asarray(inputs["input_vectors"], np.float32)
    cls = np.asarray(inputs["clstoken_scales"], np.float32)
    rois = np.asarray(inputs["rois"], np.int32)
    wqT = np.asarray(inputs["Wq"], np.float32).T
    wkT = np.asarray(inputs["Wk"], np.float32).T
    wvT = np.asarray(inputs["Wv"], np.float32).T
    bq = np.asarray(inputs["bq"], np.float32)
    bk = np.asarray(inputs["bk"], np.float32)
    bv = np.asarray(inputs["bv"], np.float32)
    has_bq = bool(np.any(bq))
    has_bk = bool(np.any(bk))
    has_bv = bool(np.any(bv))

    # rois per batch, sorted by start; padded by repeating the last roi
    ords, counts = [], []
    for b in range(B):
        sel = np.nonzero(rois[:, 0] == b)[0]
        if len(sel):
            sel = sel[np.argsort(rois[sel, 1], kind="stable")]
        ords.append(sel)
        counts.append(len(sel))
    npad = max(max(counts), 1)
    cols = npad * NBK

    # per-core channel selections and weight slices
    colsel = [np.concatenate([np.arange(h * 128, (h + 1) * 128) for h in hs]) for hs in HSETS]
    w_slices = []
    for hh in range(2):
        cs = colsel[hh]
        w_slices.append(
            (
                np.ascontiguousarray(wqT[:, cs]).astype(ml_dtypes.bfloat16),
                np.ascontiguousarray(wkT[:, cs]).astype(ml_dtypes.bfloat16),
                np.ascontiguousarray(wvT[:, cs]).astype(ml_dtypes.bfloat16),
                np.ascontiguousarray(bq[cs]),
                np.ascontiguousarray(bk[cs]),
                np.ascontiguousarray(bv[cs]),
            )
        )

    # conv weights: [jl(2), ctl(2), bin(10)] x [p=128 in-ch, 256 out] zero-padded
    wt_cores, cb_cores = [], []
    for hh in range(2):
        wt = np.zeros((2, 2, NBK, 128, 256), np.float32)
        cbs = []
        for jl, (nb, off) in enumerate(SCK[hh]):
            cwj = np.asarray(inputs[f"conv_w{nb}"], np.float32)      # [o 256, c 256, i nb]
            a = cwj.transpose(1, 2, 0)                               # [c, i, o]
            for ctl in range(2):
                for i in range(nb):
                    wt[jl, ctl, off + i] = a[ctl * 128 : (ctl + 1) * 128, i, :]
            cbs.append(np.asarray(inputs[f"conv_b{nb}"], np.float32))
        wt_cores.append(np.ascontiguousarray(wt.reshape(4 * NBK * 128, 256)).astype(ml_dtypes.bfloat16))
        cb_cores.append(np.ascontiguousarray(np.broadcast_to(np.concatenate(cbs), (npad, CH))))

    ncf = build_fused(npad, has_bq, has_bk, has_bv)
    in_maps = []
    bin_meta = []  # per half-set: (i_frac_lo[nb], ...) via exact ints below
    for core in range(8):
        b, hh = core // 2, core % 2
        sel = ords[b]
        rs = rois[sel] if len(sel) else np.array([[b, 0, 16]], np.int32)
        rs_p = np.concatenate([rs, np.repeat(rs[-1:], npad - len(rs), axis=0)])
        s = rs_p[:, 1].astype(np.int64)
        e = rs_p[:, 2].astype(np.int64)
        L = e - s
        bsa = np.zeros((npad, NBK), np.int64)
        bea = np.zeros((npad, NBK), np.int64)
        for nb, off in SCK[hh]:
            i = np.arange(nb)
            bsa[:, off : off + nb] = s[:, None] + (i[None, :] * L[:, None]) // nb
            bea[:, off : off + nb] = s[:, None] - (-(i[None, :] + 1) * L[:, None]) // nb
        cnt = np.maximum(bea - bsa, 1).astype(np.float32)
        wqh, wkh, wvh, bqh, bkh, bvh = w_slices[hh]
        m = {
            "xb": np.ascontiguousarray(iv[b]).astype(ml_dtypes.bfloat16),
            "wqh": wqh, "wkh": wkh, "wvh": wvh,
            "clsb": np.ascontiguousarray(cls[b][:, None]).astype(ml_dtypes.bfloat16),
            "bsr": np.ascontiguousarray(bsa.reshape(1, cols).astype(np.float32)),
            "ber": np.ascontiguousarray(bea.reshape(1, cols).astype(np.float32)),
            "invr": np.ascontiguousarray((1.0 / cnt).reshape(1, cols)),
            "wt": wt_cores[hh],
            "cbh": cb_cores[hh],
        }
        if has_bq:
            m["bqr"] = np.ascontiguousarray(bqh[None, :])
        if has_bk:
            m["bkc"] = np.ascontiguousarray(bkh.reshape(HC, 128).T)
        if has_bv:
            m["bvr"] = np.ascontiguousarray(np.broadcast_to(bvh, (128, CH)))
        in_maps.append(m)

    r = bass_utils.run_bass_kernel_spmd(ncf, in_maps, core_ids=list(range(8)), trace=TRACE)
    if r.exec_time_ns:
        LAST_EXEC_NS += r.exec_time_ns
    LAST_RES.append(r)

    final = np.empty((NROI, D), np.float32)
    for core in range(8):
        b, hh = core // 2, core % 2
        sel = ords[b]
        if not len(sel):
            continue
        o = r.results[core]["out"]  # [npad, 512]
        for jl in range(2):
            lo, hi = OUTCOL[hh][jl]
            final[sel, lo:hi] = o[: len(sel), jl * 256 : (jl + 1) * 256]
    return final
